# revision 50
# baseline (speedup 1.0000x reference)
"""AttentionalPropagation (SuperGlue-style GNN message passing) on 8 trn2 cores.

Problem (hardcoded): B=2, D=256, N=M=4096, H=4 heads, head dim 64.
  q = P_q(x); k = P_k(source); v = P_v(source)      (bottleneck 1x1 convs D->D/8->D)
  msg = attn(q, k, v); merged = P_m(msg)            (per-head softmax over M)
  out = Conv(relu(BN(Conv(cat[x, merged]))))        (512->64->256)

Sharding: 8 cores = (batch b in {0,1}) x (query chunk of 1024).  Each core
computes k1/v1 for its full batch row and attention + MLP for its 1024 query
columns.  Weights replicated.  No collectives.

Algebraic folds (host side):
  * scores = k1e^T C_h q1e with C_h = Ak'_h @ Aq'_h^T (33x33), where Ak'/Aq'
    are the bias-extended per-head second-projection blocks with the
    first-projection biases folded into their ones-rows.  Neither Wk2 nor Wq2
    ever runs on device.
  * Wv2 never runs on device either: msg_h = Wv2_h (sum_m p_m v1e_m)/denom,
    and Wv2/Wm1/Wm2 + all biases collapse into wmm_h = Wm2 @ [bias_h | W_h]
    applied directly to the normalized (33-row) v1-space message.
  * BN scale folds into Wp1, BN shift + conv bias into the relu bias.

Attention layout: scores computed transposed (keys m on partitions, queries n
free).  kv1_all rows: 0-31 k1(raw), 32-63 v1(raw), 64 ones, 65-127 zero.
v1e^T (per 128-m-chunk, [ones|v1] columns) is produced by a tiny selector
matmul; the msg matmul contracts it against exp(scores) in fp8 with
perf_mode=DoubleRow (virtual K=256), row 0 of the msg PSUM = softmax denom.

exp runs split across two engines: ACT (exact, fp8 out) and DVE (Schraudolph
bit-trick: bits8 = round(s/ln2 + 56) written as int8, bitcast to fp8e4).

HAM note: all hot matmuls are full-K (128 partitions); the DR msg matmuls are
128-partition too.  Small-K matmuls (merge/mlp/q) ride the warm clock.
"""

import numpy as np

import concourse.bass as bass
import concourse.mybir as mybir
import concourse.tile as tile
from concourse import bacc, bass_utils

B, D, N, M, H = 2, 256, 4096, 4096, 4
DIM = D // H       # 64
D8 = D // 8        # 32
TD = 2 * D         # 512
TD8 = TD // 8      # 64
BN_EPS = 1e-5
NCORES = 8
NCHUNK = N // 4    # query columns per core
NT = 512           # n tile (PSUM bank = 512 fp32)
NTILES = NCHUNK // NT          # 2
MT = 512           # source m tile for k/v projection stage
MTILES = M // MT               # 8
MC = 128           # m chunk (scores PSUM partition dim)
MCHUNKS = M // MC              # 32
NSTEP = MCHUNKS // 2           # 16 steps (2 chunks each) per (nt, h)
F32 = mybir.dt.float32
F32R = mybir.dt.float32r
BF16 = mybir.dt.bfloat16
FP8 = mybir.dt.float8e4
I8 = mybir.dt.int8
AF = mybir.ActivationFunctionType
ALU = mybir.AluOpType

WARMUP_MMS = 8
# Schraudolph constants for fp8e4(m3) bits of exp(0.125*s):
# bits = s*(0.125*8/ln2) + 8*7 - 0.458 + 0.5(trunc comp)
EXP_A = 0.125 * 8.0 / np.log(2.0)
EXP_B = 56.0 - 0.458 + 0.5

# ---- early bf16 consts (stage B critical) ----
CE_WQ = 0            # [128, 2, 32]
CE_IV = 64           # [65, 34]
CE_CHT = 98          # [33, H, 128]
CE_END = 610
# ---- late bf16 consts (merge/mlp, needed ~90us in) ----
CB_WMM = 0           # [33, H, 2, 128]
CB_WP1M = 1024       # [128, 2, 64]
CB_WP1X = 1152       # [128, 2, 64]
CB_END = 1280
# ---- f32 const pack offsets (late) ----
CF_WP2 = 0           # [65, 2, 128]
CF_BE1 = 256         # [64, 1]
CF_END = 257


def dve_step(nt, h, bi):
    """Which exp steps run on DVE (Schraudolph) instead of ACT."""
    if h == 0:
        return bi in (3, 5, 7, 9, 11, 13)
    return bi in (2, 4, 6, 8, 10, 12, 14)


def build_body(ctx, tc: tile.TileContext, io):
    nc = tc.nc
    x_d = io["x_chunk"]          # [128, 2, NCHUNK]  (partition, channel-chunk, n)
    src_d = io["source_b"]       # [128, 2, M]
    out_d = io["out_chunk"]      # [2, 128, NCHUNK]

    consts = ctx.enter_context(tc.tile_pool(name="consts", bufs=1))
    big = ctx.enter_context(tc.tile_pool(name="big", bufs=1))
    srcp = ctx.enter_context(tc.tile_pool(name="srcp", bufs=4))
    ep = ctx.enter_context(tc.tile_pool(name="ep", bufs=4))
    nrm = ctx.enter_context(tc.tile_pool(name="nrm", bufs=4))

    # ---- const DMAs first: gpsimd FIFO must not stall them ----
    w8 = consts.tile([128, 2, 64], FP8)   # [ct][k-cols 0-31 | v-cols 32-63]
    ce = consts.tile([128, CE_END], BF16)
    cb = consts.tile([128, CB_END], BF16)
    cf = consts.tile([128, CF_END], F32R)

    cht = lambda h: ce[:, CE_CHT + h * 128: CE_CHT + (h + 1) * 128]
    iv_ap = ce[:, CE_IV: CE_IV + 34]
    wmm = lambda h, ct: cb[0:33, CB_WMM + (h * 2 + ct) * 128: CB_WMM + (h * 2 + ct + 1) * 128]
    wp1m = lambda ct: cb[:, CB_WP1M + ct * TD8: CB_WP1M + (ct + 1) * TD8]
    wq1 = lambda ct: ce[:, CE_WQ + ct * D8: CE_WQ + (ct + 1) * D8]
    wp1x = lambda ct: cb[:, CB_WP1X + ct * TD8: CB_WP1X + (ct + 1) * TD8]
    wp2 = lambda ct: cf[0:TD8 + 1, CF_WP2 + ct * 128: CF_WP2 + (ct + 1) * 128]
    be1_ap = cf[0:TD8, CF_BE1: CF_BE1 + 1].bitcast(F32)

    # ---- persistent activations ----
    kv1_all = big.tile([128, M], BF16)   # rows 0-31 k1, 32-63 v1, 64 ones, 65+ 0
    v1t = big.tile([128, MCHUNKS, 48], FP8)   # [ones|v1e]^T per m chunk, 34 used
    # input DMAs: host pre-transposes x/src to partition-major so each is a
    # single strided DMA; src lands in 4 big chunks (fewer sync-issue slots)
    x_sb = big.tile([128, 2, NCHUNK], BF16)
    src_tiles = []
    for c in range(4):
        t = srcp.tile([128, 2, 2, MT], FP8, tag="src", name=f"src{c}")
        src_tiles.append(t)
    nc.sync.dma_start(out=w8, in_=io["c8"])
    nc.sync.dma_start(out=src_tiles[0], in_=src_d[:, 0:2, :, :])
    nc.sync.dma_start(out=ce, in_=io["ce"])
    nc.sync.dma_start(out=src_tiles[1], in_=src_d[:, 2:4, :, :])
    nc.sync.dma_start(out=x_sb[:, :, 0:NT], in_=x_d[:, :, 0:NT])
    nc.sync.dma_start(out=x_sb[:, :, NT:NCHUNK], in_=x_d[:, :, NT:NCHUNK])
    nc.sync.dma_start(out=src_tiles[2], in_=src_d[:, 4:6, :, :])
    nc.sync.dma_start(out=src_tiles[3], in_=src_d[:, 6:8, :, :])
    nc.sync.dma_start(out=cf, in_=io["cf"])
    nc.sync.dma_start(out=cb, in_=io["cb"])
    qh_sb = big.tile([128, H, NCHUNK], BF16)          # C_h q1e, rows 33+ zero
    q1 = big.tile([128, NCHUNK], BF16)    # 0-31 q1, 32 ones, 33+ zero
    msg_sb = big.tile([33, H, NCHUNK], BF16)          # row 0 = 1, 1-32 mv1n
    mm_sb = big.tile([128, 2, NCHUNK], BF16)          # merged msg (mlp input)
    h1 = big.tile([TD8 + 1, NCHUNK], F32R)            # relu(BN(.)), row 64 ones
    out_sb = big.tile([128, 2, NCHUNK], F32)

    # ---- PE warm-up; scratch memsets are tiny so it starts early ----
    wza = consts.tile([128, 128], BF16)
    wzb = consts.tile([128, NT], BF16)
    nc.vector.memset(wza, 0.0)
    nc.vector.memset(wzb, 0.0)
    ppw = tc.tile_pool(name="ppw", bufs=2, space="PSUM")
    ppw_pool = ppw.__enter__()
    for i in range(WARMUP_MMS):
        pw = ppw_pool.tile([128, NT], F32, tag="pw", name="pw")
        nc.tensor.matmul(pw, wza, wzb, start=True, stop=True)
    ppw.__exit__(None, None, None)

    # ---- memsets (ones rows first: q1/h1 gate early compute) ----
    nc.vector.memset(q1[32:64, :], 0.0)
    nc.vector.memset(q1[64:128, :], 0.0)
    nc.gpsimd.memset(q1[32:33, :], 1.0)
    nc.gpsimd.memset(h1[TD8:TD8 + 1, :].bitcast(F32), 1.0)
    nc.vector.memset(kv1_all[64:128, :], 0.0)
    nc.gpsimd.memset(kv1_all[64:65, :], 1.0)

    # ---- stage B: k1/v1 over full M + v1e transpose, SW-pipelined ----
    ppb = tc.tile_pool(name="ppb", bufs=3, space="PSUM")
    ppb_pool = ppb.__enter__()

    def emit_kv1(mt):
        ms = mt * MT
        src = src_tiles[mt // 2][:, mt % 2, :, :]
        ps1 = ppb_pool.tile([64, MT], F32, tag="ps1", name="ps1")
        nc.tensor.matmul(ps1, w8, src, start=True, stop=True,
                         perf_mode=mybir.MatmulPerfMode.DoubleRow)
        if mt % 2 == 0:
            nc.scalar.copy(out=kv1_all[0:64, ms:ms + MT], in_=ps1)
        else:
            nc.vector.tensor_copy(out=kv1_all[0:64, ms:ms + MT], in_=ps1)

    def emit_v1t(p):
        ms8 = p * 8
        psv = ppb_pool.tile([128, 8, 34], F32, tag="psv", name="psv")
        for j in range(8):
            mc = ms8 + j
            nc.tensor.matmul(psv[:, j, :], kv1_all[:, mc * MC:(mc + 1) * MC],
                             iv_ap, start=True, stop=True)
        nc.scalar.copy(out=v1t[:, ms8:ms8 + 8, 0:34], in_=psv)

    def emit_q1(nt):
        ns = nt * NT
        psq = ppb_pool.tile([D8, NT], F32, tag="ps1", name="psq")
        nc.tensor.matmul(psq, wq1(0), x_sb[:, 0, ns:ns + NT], start=True, stop=False)
        nc.tensor.matmul(psq, wq1(1), x_sb[:, 1, ns:ns + NT], start=False, stop=True)
        nc.vector.tensor_copy(out=q1[0:D8, ns:ns + NT], in_=psq)

    def emit_qh(h, nt, act):
        ns = nt * NT
        psq2 = ppb_pool.tile([128, NT], F32, tag="psv", name="psq2")
        nc.tensor.matmul(psq2, cht(h), q1[:, ns:ns + NT], start=True, stop=True)
        if act:
            nc.scalar.copy(out=qh_sb[:, h, ns:ns + NT], in_=psq2)
        else:
            nc.vector.tensor_copy(out=qh_sb[:, h, ns:ns + NT], in_=psq2)

    # fully interleaved stage B: kv1 stream + v1t transposes + the whole q
    # path, alternating copy engines so neither ACT nor DVE serializes it.
    emit_kv1(0)
    emit_kv1(1)
    emit_kv1(2)
    emit_kv1(3)
    emit_v1t(0)
    emit_kv1(4)
    emit_kv1(5)
    emit_v1t(1)
    emit_q1(0)
    emit_q1(1)
    emit_kv1(6)
    emit_kv1(7)
    emit_v1t(2)
    emit_qh(0, 0, act=True)
    emit_qh(0, 1, act=False)
    emit_v1t(3)
    emit_qh(1, 0, act=True)
    emit_qh(1, 1, act=False)
    emit_qh(2, 0, act=True)
    emit_qh(2, 1, act=False)
    emit_qh(3, 0, act=True)
    emit_qh(3, 1, act=False)

    ppb.__exit__(None, None, None)

    # ---- attention: flat pipeline over (nt, h, bi), lookahead 2 ----
    pps = ctx.enter_context(tc.tile_pool(name="pps", bufs=3, space="PSUM"))
    ppm = ctx.enter_context(tc.tile_pool(name="ppm", bufs=2, space="PSUM"))

    def emit_bridge_warm(dep_ap, n, off):
        # tiny DVE copy whose input is `dep_ap`: delays the following warm
        # MMs until that data exists, filling the PE hole right after it
        nc.vector.tensor_copy(out=wzb[0:1, off:off + 8], in_=dep_ap)
        emit_warm_mms(n)

    def emit_warm_mms(n, rhs=None):
        # dummy full-K matmuls: hold the HAM clock at 8/8 through thin spots.
        # Results are discarded.  Passing a just-written rhs tile delays
        # execution until that tile lands, so the warm MMs fill the
        # dependency-wait hole instead of running immediately.
        pw = ppm.tile([128, NT], F32, tag="pm", name="pwarm")
        r = wzb if rhs is None else rhs
        for _ in range(n):
            nc.tensor.matmul(pw[:, 0:r.shape[-1]], wza, r, start=True, stop=True)

    emit_warm_mms(3, rhs=qh_sb[:, 0, 0:NT])
    emit_warm_mms(3, rhs=qh_sb[:, 0, NT:2 * NT])

    def emit_scores(nt, h, bi):
        ns = nt * NT
        ps = pps.tile([128, 2, NT], F32, tag="ps", name="ps")
        for j in range(2):
            mc = bi * 2 + j
            nc.tensor.matmul(ps[:, j, :], kv1_all[:, mc * MC:(mc + 1) * MC],
                             qh_sb[:, h, ns:ns + NT], start=True, stop=True)
        e = ep.tile([128, 2, NT], FP8, tag="e", name="e")
        if dve_step(nt, h, bi):
            nc.vector.tensor_scalar(
                out=e[:, :, :].bitcast(I8), in0=ps, scalar1=float(EXP_A),
                scalar2=float(EXP_B), op0=ALU.mult, op1=ALU.add)
        else:
            nc.scalar.activation(out=e, in_=ps, func=AF.Exp, scale=0.125)
        return e

    def emit_norm(pm, h, ns, bridge=False):
        if not bridge:
            rec = nrm.tile([1, NT], F32, tag="rec", name="rec")
            nc.vector.reciprocal_approx_fast(out=rec, in_=pm[0:1, :])
            bc = nrm.tile([33, NT], F32, tag="bc", name="bc")
            nc.gpsimd.partition_broadcast(bc, rec)
            nc.vector.tensor_mul(out=msg_sb[0:33, h, ns:ns + NT],
                                 in0=pm[0:33, :], in1=bc)
            return
        # final head: halve the norm so the tail merge starts ~1.2us earlier
        HT = NT // 2
        for half in range(2):
            o = half * HT
            rec = nrm.tile([1, HT], F32, tag="rec", name="rec")
            nc.vector.reciprocal_approx_fast(out=rec, in_=pm[0:1, o:o + HT])
            if half == 0:
                emit_bridge_warm(rec[0:1, 0:8].bitcast(BF16)[0:1, 0:8], 3, 0)
            bc = nrm.tile([33, HT], F32, tag="bc", name="bc")
            nc.gpsimd.partition_broadcast(bc, rec)
            nc.vector.tensor_mul(out=msg_sb[0:33, h, ns + o:ns + o + HT],
                                 in0=pm[0:33, o:o + HT], in1=bc)


    def make_merge_pieces(nt):
        # nt0's merge+mlp split into 4 pieces, one per nt1 head boundary.
        # Each allocates exactly one 1-bank ppm tile, interleaving cleanly
        # with the pm rotation so the scores pipeline is never disturbed.
        ns = nt * NT
        st = {}

        def p_ct(ct):
            def f():
                t = ppm.tile([128, NT], F32, tag="pm", name="psm2n")
                for h in range(H):
                    nc.tensor.matmul(t, wmm(h, ct), msg_sb[:, h, ns:ns + NT],
                                     start=(h == 0), stop=(h == H - 1))
                nc.vector.tensor_copy(out=mm_sb[:, ct, ns:ns + NT], in_=t)
            return f

        def p_psh():
            t = ppm.tile([128, NT], F32, tag="pm", name="pshn")
            st["ph"] = t
            psh = t[0:TD8, :]
            nc.tensor.matmul(psh, wp1x(0), x_sb[:, 0, ns:ns + NT], start=True, stop=False)
            nc.tensor.matmul(psh, wp1x(1), x_sb[:, 1, ns:ns + NT], start=False, stop=False)
            nc.tensor.matmul(psh, wp1m(0), mm_sb[:, 0, ns:ns + NT], start=False, stop=False)
            nc.tensor.matmul(psh, wp1m(1), mm_sb[:, 1, ns:ns + NT], start=False, stop=True)
            nc.scalar.activation(out=h1[0:TD8, ns:ns + NT], in_=psh, func=AF.Relu,
                                 bias=be1_ap)

        def p_out():
            t = ppm.tile([128, NT], F32, tag="pm", name="pson")
            for ct, tt in ((0, t), (1, st["ph"])):
                nc.tensor.matmul(tt[:, :], wp2(ct), h1[:, ns:ns + NT],
                                 start=True, stop=True)
                nc.vector.tensor_copy(out=out_sb[:, ct, ns:ns + NT], in_=tt)
                nc.sync.dma_start(out=out_d[ct, :, ns:ns + NT],
                                  in_=out_sb[:, ct, ns:ns + NT])

        return [p_ct(0), p_ct(1), p_psh, p_out]

    merge_pieces = []

    def emit_merge_mlp(nt, warm=False):
        # the tail merge runs half-width (256-col) chains for latency;
        # mid-attention merges run full-width to minimize pps disturbance
        nhalf = 2 if warm else 1
        HT = NT // nhalf
        # bridge: a tiny DVE copy dependent on the last norm makes the
        # following warm MMs fire exactly when the PE would go idle
        nc.vector.tensor_copy(out=wzb[0:33, 0:8],
                              in_=msg_sb[:, H - 1, nt * NT:nt * NT + 8])
        emit_warm_mms(5)
        for half in range(nhalf):
            ns = nt * NT + half * HT
            psm2 = pps.tile([128, 2, HT], F32, tag="ps", name="psm2")
            for ct in range(2):
                for h in range(H):
                    nc.tensor.matmul(psm2[:, ct, :], wmm(h, ct),
                                     msg_sb[:, h, ns:ns + HT],
                                     start=(h == 0), stop=(h == H - 1))
                nc.vector.tensor_copy(out=mm_sb[:, ct, ns:ns + HT],
                                      in_=psm2[:, ct, :])
                if warm:
                    emit_warm_mms(2, rhs=mm_sb[:, ct, ns:ns + HT])
            phb = pps.tile([128, 2, HT], F32, tag="ps", name="phb")
            psh = phb[0:TD8, 0, :]
            nc.tensor.matmul(psh, wp1x(0), x_sb[:, 0, ns:ns + HT], start=True, stop=False)
            nc.tensor.matmul(psh, wp1x(1), x_sb[:, 1, ns:ns + HT], start=False, stop=False)
            nc.tensor.matmul(psh, wp1m(0), mm_sb[:, 0, ns:ns + HT], start=False, stop=False)
            nc.tensor.matmul(psh, wp1m(1), mm_sb[:, 1, ns:ns + HT], start=False, stop=True)
            nc.scalar.activation(out=h1[0:TD8, ns:ns + HT], in_=psh, func=AF.Relu,
                                 bias=be1_ap)
            if warm:
                emit_bridge_warm(h1[0:1, ns:ns + 8].bitcast(BF16)[0:1, 0:8],
                                 2, 8 + 8 * half)
            for ct in range(2):
                pso = phb[:, 1 - ct, :]
                nc.tensor.matmul(pso, wp2(ct), h1[:, ns:ns + HT], start=True, stop=True)
                nc.vector.tensor_copy(out=out_sb[:, ct, ns:ns + HT], in_=pso)
                nc.sync.dma_start(out=out_d[ct, :, ns:ns + HT],
                                  in_=out_sb[:, ct, ns:ns + HT])

    seq = [(nt, h, bi) for nt in range(NTILES) for h in range(H)
           for bi in range(NSTEP)]
    pend = {}
    pm = None

    def emit_msg(idx):
        nonlocal pm
        nt, h, bi = seq[idx]
        if bi == 0:
            pm = ppm.tile([33, NT], F32, tag="pm", name="pm")
        e = pend.pop(idx)
        nc.tensor.matmul(pm, v1t[:, 2 * bi: 2 * bi + 2, 0:33], e,
                         start=(bi == 0), stop=(bi == NSTEP - 1),
                         perf_mode=mybir.MatmulPerfMode.DoubleRow)
        if bi == NSTEP - 1:
            emit_norm(pm, h, nt * NT,
                      bridge=(nt == NTILES - 1 and h == H - 1))
            if nt == 0 and h == H - 1:
                merge_pieces[:] = make_merge_pieces(0)
            elif nt == NTILES - 1:
                merge_pieces[h]()
                if h == H - 1:
                    emit_merge_mlp(nt, warm=True)

    LOOK = 2
    for idx, step in enumerate(seq):
        pend[idx] = emit_scores(*step)
        if idx >= LOOK:
            emit_msg(idx - LOOK)
    for idx in range(len(seq) - LOOK, len(seq)):
        emit_msg(idx)


def build_program():
    nc = bacc.Bacc("TRN2", target_bir_lowering=False, debug=False)
    io = {}
    io["x_chunk"] = nc.dram_tensor("x_chunk", [128, 2, NCHUNK], BF16,
                                   kind="ExternalInput").ap()
    io["source_b"] = nc.dram_tensor("source_b", [128, MTILES, 2, MT], FP8,
                                    kind="ExternalInput").ap()
    io["c8"] = nc.dram_tensor("c8", [128, 2, 64], FP8,
                              kind="ExternalInput").ap()
    io["ce"] = nc.dram_tensor("ce", [128, CE_END], BF16, kind="ExternalInput").ap()
    io["cb"] = nc.dram_tensor("cb", [128, CB_END], BF16, kind="ExternalInput").ap()
    io["cf"] = nc.dram_tensor("cf", [128, CF_END], F32R, kind="ExternalInput").ap()
    io["out_chunk"] = nc.dram_tensor(
        "out_chunk", [2, 128, NCHUNK], F32, kind="ExternalOutput").ap()
    from contextlib import ExitStack
    with tile.TileContext(nc) as tc, ExitStack() as ctx:
        build_body(ctx, tc, io)
    nc.compile()
    return nc


def prep_weights(i):
    """Host-side folds; see module docstring."""
    import ml_dtypes
    bf = ml_dtypes.bfloat16
    f = np.float32
    a = {k: np.asarray(v, dtype=f) for k, v in i.items()}
    # head-contiguous channel permutation: c' = h*64+d  <- c = 4*d+h
    perm = (np.arange(H)[:, None] + H * np.arange(DIM)[None, :]).reshape(-1)

    def w1t(w):       # [D8, D] -> [128, 2*D8] (chunk-major)
        return np.ascontiguousarray(
            w.T.reshape(2, 128, D8).swapaxes(0, 1).reshape(128, 2 * D8))

    cep = np.zeros((128, CE_END), np.float64)
    cbp = np.zeros((128, CB_END), np.float64)
    cfp = np.zeros((128, CF_END), np.float64)

    cep[:, CE_WQ:CE_WQ + 64] = w1t(a["Wq1"])
    c8 = np.zeros((128, 2, 64), np.float32)
    c8[:, :, 0:32] = w1t(a["Wk1"]).reshape(128, 2, D8)
    c8[:, :, 32:64] = w1t(a["Wv1"]).reshape(128, 2, D8)

    # cht: C_h = Ak'_h @ Aq'_h^T with first-proj biases folded into ones-rows
    wq2e = np.concatenate([a["Wq2"][perm].T, a["bq2"][perm][None, :]], 0)  # [33, 256]
    wk2e = np.concatenate([a["Wk2"][perm].T, a["bk2"][perm][None, :]], 0)
    for h in range(H):
        hs = slice(h * DIM, (h + 1) * DIM)
        Ak = wk2e[:, hs].astype(np.float64).copy()      # [33, 64]
        Aq = wq2e[:, hs].astype(np.float64).copy()
        Ak[32, :] += a["bk1"].astype(np.float64) @ Ak[0:32, :]
        Aq[32, :] += a["bq1"].astype(np.float64) @ Aq[0:32, :]
        C = Ak @ Aq.T                                   # [33(k-space), 33(q-space)]
        # cht[kq, h, p]: p<32 -> C[p, kq]; p=64 -> C[32, kq]
        blk = np.zeros((33, 128), np.float64)
        blk[:, 0:32] = C[0:32, :].T
        blk[:, 64] = C[32, :]
        cep[0:33, CE_CHT + h * 128: CE_CHT + (h + 1) * 128] = blk

    # iv: [65, 34] selector: col 0 <- ones row (64), col 1+j <- v1 row (32+j)
    iv = np.zeros((65, 34), np.float64)
    iv[64, 0] = 1.0
    for j in range(32):
        iv[32 + j, 1 + j] = 1.0
    cep[0:65, CE_IV:CE_IV + 34] = iv

    # wmm: mm = sum_h Wm2 @ [bias_h | W_h] @ msgv_h  (+ Wm2@bm1 + bm2 on h=0 col0)
    Wm2 = a["Wm2"].astype(np.float64)                   # [256, 32]
    for h in range(H):
        hs = perm[h * DIM:(h + 1) * DIM]
        Wm1p = a["Wm1"][:, hs].astype(np.float64)       # [32, 64]
        Wv2p = a["Wv2"][hs, :].astype(np.float64)       # [64, 32]
        bv2p = a["bv2"][hs].astype(np.float64)          # [64]
        W_h = Wm1p @ Wv2p                               # [32, 32]
        bias_h = Wm1p @ (bv2p + Wv2p @ a["bv1"].astype(np.float64))
        if h == 0:
            bias_h = bias_h + a["bm1"].astype(np.float64)
        Mfull = np.concatenate([bias_h[:, None], W_h], 1)   # [32, 33] (m1-space)
        M2 = Wm2 @ Mfull                                # [256, 33]
        if h == 0:
            M2[:, 0] += a["bm2"].astype(np.float64)
        for ct in range(2):
            cbp[0:33, CB_WMM + (h * 2 + ct) * 128: CB_WMM + (h * 2 + ct + 1) * 128] = \
                M2[ct * 128:(ct + 1) * 128, :].T

    # BN scale folds into Wp1; shift+bias into relu bias
    g1s = (a["g1"] / np.sqrt(np.float64(1.0) + np.float64(BN_EPS))).astype(np.float64)
    wp1x = a["Wp1"][:, 0:D].astype(np.float64) * g1s[:, None]     # [64, 256]
    wp1m = a["Wp1"][:, D:TD].astype(np.float64) * g1s[:, None]
    cbp[:, CB_WP1X:CB_WP1X + 128] = \
        wp1x.T.reshape(2, 128, TD8).swapaxes(0, 1).reshape(128, 128)
    cbp[:, CB_WP1M:CB_WP1M + 128] = \
        wp1m.T.reshape(2, 128, TD8).swapaxes(0, 1).reshape(128, 128)
    cfp[0:TD8, CF_BE1] = a["be1"].astype(np.float64) + g1s * a["bp1"].astype(np.float64)

    wp2t = np.concatenate(
        [a["Wp2"].T.reshape(TD8, 2, 128), a["bp2"].reshape(1, 2, 128)], 0)  # [65,2,128]
    cfp[0:TD8 + 1, CF_WP2:CF_WP2 + 256] = wp2t.reshape(TD8 + 1, 256)

    return {"ce": np.ascontiguousarray(cep.astype(f).astype(bf)),
            "cb": np.ascontiguousarray(cbp.astype(f).astype(bf)),
            "cf": np.ascontiguousarray(cfp.astype(f)),
            "c8": np.ascontiguousarray(c8.astype(ml_dtypes.float8_e4m3))}


_NC_CACHE = None


def _get_nc():
    global _NC_CACHE
    if _NC_CACHE is None:
        _NC_CACHE = build_program()
    return _NC_CACHE


def make_in_maps(inputs):
    import ml_dtypes
    w = prep_weights(inputs)
    x = np.ascontiguousarray(np.asarray(inputs["x"], np.float32))
    src = np.ascontiguousarray(np.asarray(inputs["source"], np.float32))
    in_maps = []
    for c in range(NCORES):
        b, ns = c // 4, (c % 4) * NCHUNK
        m = dict(w)
        m["x_chunk"] = np.ascontiguousarray(
            x[b].reshape(2, 128, N)[:, :, ns:ns + NCHUNK].transpose(1, 0, 2)
            ).astype(ml_dtypes.bfloat16)
        m["source_b"] = np.ascontiguousarray(
            src[b].reshape(2, 128, MTILES, MT).transpose(1, 2, 0, 3)).astype(
            ml_dtypes.float8_e4m3)
        in_maps.append(m)
    return in_maps


def assemble_out(results):
    out = np.empty((B, D, N), np.float32)
    for c in range(NCORES):
        b, ns = c // 4, (c % 4) * NCHUNK
        out[b].reshape(2, 128, N)[:, :, ns:ns + NCHUNK] = (
            results[c]["out_chunk"])
    return out


def kernel(**inputs):
    nc = _get_nc()
    res = bass_utils.run_bass_kernel_spmd(
        nc, make_in_maps(inputs), core_ids=list(range(NCORES)))
    return assemble_out(res.results)


# revision 51
# speedup vs baseline: 1.0248x; 1.0248x over previous
"""AttentionalPropagation (SuperGlue-style GNN message passing) on 8 trn2 cores.

Problem (hardcoded): B=2, D=256, N=M=4096, H=4 heads, head dim 64.
  q = P_q(x); k = P_k(source); v = P_v(source)      (bottleneck 1x1 convs D->D/8->D)
  msg = attn(q, k, v); merged = P_m(msg)            (per-head softmax over M)
  out = Conv(relu(BN(Conv(cat[x, merged]))))        (512->64->256)

Sharding: 8 cores = (batch b in {0,1}) x (query chunk of 1024).  Each core
computes k1/v1 for its full batch row and attention + MLP for its 1024 query
columns.  Weights replicated.  No collectives.

Algebraic folds (host side):
  * scores = k1e^T C_h q1e with C_h = Ak'_h @ Aq'_h^T (33x33), where Ak'/Aq'
    are the bias-extended per-head second-projection blocks with the
    first-projection biases folded into their ones-rows.  Neither Wk2 nor Wq2
    ever runs on device.
  * Wv2 never runs on device either: msg_h = Wv2_h (sum_m p_m v1e_m)/denom,
    and Wv2/Wm1/Wm2 + all biases collapse into wmm_h = Wm2 @ [bias_h | W_h]
    applied directly to the normalized (33-row) v1-space message.
  * BN scale folds into Wp1, BN shift + conv bias into the relu bias.

Attention layout: scores computed transposed (keys m on partitions, queries n
free).  kv1_all rows: 0-31 k1(raw), 32-63 v1(raw), 64 ones, 65-127 zero.
v1e^T (per 128-m-chunk, [ones|v1] columns) is produced by a tiny selector
matmul; the msg matmul contracts it against exp(scores) in fp8 with
perf_mode=DoubleRow (virtual K=256), row 0 of the msg PSUM = softmax denom.

exp runs split across two engines: ACT (exact, fp8 out) and DVE (Schraudolph
bit-trick: bits8 = round(s/ln2 + 56) written as int8, bitcast to fp8e4).

HAM note: all hot matmuls are full-K (128 partitions); the DR msg matmuls are
128-partition too.  Small-K matmuls (merge/mlp/q) ride the warm clock.
"""

import numpy as np

import concourse.bass as bass
import concourse.mybir as mybir
import concourse.tile as tile
from concourse import bacc, bass_utils

B, D, N, M, H = 2, 256, 4096, 4096, 4
DIM = D // H       # 64
D8 = D // 8        # 32
TD = 2 * D         # 512
TD8 = TD // 8      # 64
BN_EPS = 1e-5
NCORES = 8
NCHUNK = N // 4    # query columns per core
NT = 512           # n tile (PSUM bank = 512 fp32)
NTILES = NCHUNK // NT          # 2
MT = 512           # source m tile for k/v projection stage
MTILES = M // MT               # 8
MC = 128           # m chunk (scores PSUM partition dim)
MCHUNKS = M // MC              # 32
NSTEP = MCHUNKS // 2           # 16 steps (2 chunks each) per (nt, h)
F32 = mybir.dt.float32
F32R = mybir.dt.float32r
BF16 = mybir.dt.bfloat16
FP8 = mybir.dt.float8e4
I8 = mybir.dt.int8
AF = mybir.ActivationFunctionType
ALU = mybir.AluOpType

WARMUP_MMS = 8
# Schraudolph constants for fp8e4(m3) bits of exp(0.125*s):
# bits = s*(0.125*8/ln2) + 8*7 - 0.458 + 0.5(trunc comp)
EXP_A = 0.125 * 8.0 / np.log(2.0)
EXP_B = 56.0 - 0.458 + 0.5

# ---- early bf16 consts (stage B critical) ----
CE_WQ = 0            # [128, 2, 32]
CE_IV = 64           # [65, 34]
CE_CHT = 98          # [33, H, 128]
CE_END = 610
# ---- late bf16 consts (merge/mlp, needed ~90us in) ----
CB_WMM = 0           # [33, H, 2, 128]
CB_WP1M = 1024       # [128, 2, 64]
CB_WP1X = 1152       # [128, 2, 64]
CB_END = 1280
# ---- f32 const pack offsets (late) ----
CF_WP2 = 0           # [65, 2, 128]
CF_BE1 = 256         # [64, 1]
CF_END = 257


def dve_step(nt, h, bi):
    """Which exp steps run on DVE (Schraudolph) instead of ACT."""
    if h == 0:
        return bi in (3, 5, 7, 9, 11, 13)
    return bi in (2, 4, 6, 8, 10, 12, 14)


def build_body(ctx, tc: tile.TileContext, io):
    nc = tc.nc
    x_d = io["x_chunk"]          # [128, 2, NCHUNK]  (partition, channel-chunk, n)
    src_d = io["source_b"]       # [128, 2, M]
    out_d = io["out_chunk"]      # [2, 128, NCHUNK]

    consts = ctx.enter_context(tc.tile_pool(name="consts", bufs=1))
    big = ctx.enter_context(tc.tile_pool(name="big", bufs=1))
    srcp = ctx.enter_context(tc.tile_pool(name="srcp", bufs=4))
    ep = ctx.enter_context(tc.tile_pool(name="ep", bufs=4))
    nrm = ctx.enter_context(tc.tile_pool(name="nrm", bufs=4))

    # ---- const DMAs first: gpsimd FIFO must not stall them ----
    w8 = consts.tile([128, 2, 64], FP8)   # [ct][k-cols 0-31 | v-cols 32-63]
    ce = consts.tile([128, CE_END], BF16)
    cb = consts.tile([128, CB_END], BF16)
    cf = consts.tile([128, CF_END], F32R)

    cht = lambda h: ce[:, CE_CHT + h * 128: CE_CHT + (h + 1) * 128]
    iv_ap = ce[:, CE_IV: CE_IV + 34]
    wmm = lambda h, ct: cb[0:33, CB_WMM + (h * 2 + ct) * 128: CB_WMM + (h * 2 + ct + 1) * 128]
    wp1m = lambda ct: cb[:, CB_WP1M + ct * TD8: CB_WP1M + (ct + 1) * TD8]
    wq1 = lambda ct: ce[:, CE_WQ + ct * D8: CE_WQ + (ct + 1) * D8]
    wp1x = lambda ct: cb[:, CB_WP1X + ct * TD8: CB_WP1X + (ct + 1) * TD8]
    wp2 = lambda ct: cf[0:TD8 + 1, CF_WP2 + ct * 128: CF_WP2 + (ct + 1) * 128]
    be1_ap = cf[0:TD8, CF_BE1: CF_BE1 + 1].bitcast(F32)

    # ---- persistent activations ----
    kv1_all = big.tile([128, M], BF16)   # rows 0-31 k1, 32-63 v1, 64 ones, 65+ 0
    v1t = big.tile([128, MCHUNKS, 48], FP8)   # [ones|v1e]^T per m chunk, 34 used
    # input DMAs: host pre-transposes x/src to partition-major so each is a
    # single strided DMA; src lands in 4 big chunks (fewer sync-issue slots)
    x_sb = big.tile([128, 2, NCHUNK], BF16)
    src_tiles = []
    for c in range(4):
        t = srcp.tile([128, 2, 2, MT], FP8, tag="src", name=f"src{c}")
        src_tiles.append(t)
    nc.sync.dma_start(out=w8, in_=io["c8"])
    nc.sync.dma_start(out=src_tiles[0], in_=src_d[:, 0:2, :, :])
    nc.sync.dma_start(out=ce, in_=io["ce"])
    nc.sync.dma_start(out=src_tiles[1], in_=src_d[:, 2:4, :, :])
    nc.sync.dma_start(out=x_sb[:, :, 0:NT], in_=x_d[:, :, 0:NT])
    nc.sync.dma_start(out=x_sb[:, :, NT:NCHUNK], in_=x_d[:, :, NT:NCHUNK])
    nc.sync.dma_start(out=src_tiles[2], in_=src_d[:, 4:6, :, :])
    nc.sync.dma_start(out=src_tiles[3], in_=src_d[:, 6:8, :, :])
    nc.sync.dma_start(out=cf, in_=io["cf"])
    nc.sync.dma_start(out=cb, in_=io["cb"])
    qh_sb = big.tile([128, H, NCHUNK], BF16)          # C_h q1e, rows 33+ zero
    q1 = big.tile([128, NCHUNK], BF16)    # 0-31 q1, 32 ones, 33+ zero
    msg_sb = big.tile([33, H, NCHUNK], BF16)          # row 0 = 1, 1-32 mv1n
    mm_sb = big.tile([128, 2, NCHUNK], BF16)          # merged msg (mlp input)
    h1 = big.tile([TD8 + 1, NCHUNK], F32R)            # relu(BN(.)), row 64 ones
    out_sb = big.tile([128, 2, NCHUNK], F32)

    # ---- PE warm-up; scratch memsets are tiny so it starts early ----
    wza = consts.tile([128, 128], BF16)
    wzb = consts.tile([128, NT], BF16)
    nc.vector.memset(wza, 0.0)
    nc.vector.memset(wzb, 0.0)
    ppw = tc.tile_pool(name="ppw", bufs=2, space="PSUM")
    ppw_pool = ppw.__enter__()
    for i in range(WARMUP_MMS):
        pw = ppw_pool.tile([128, NT], F32, tag="pw", name="pw")
        nc.tensor.matmul(pw, wza, wzb, start=True, stop=True)
    ppw.__exit__(None, None, None)

    # ---- memsets (ones rows first: q1/h1 gate early compute) ----
    nc.vector.memset(q1[32:64, :], 0.0)
    nc.vector.memset(q1[64:128, :], 0.0)
    nc.gpsimd.memset(q1[32:33, :], 1.0)
    nc.gpsimd.memset(h1[TD8:TD8 + 1, :].bitcast(F32), 1.0)
    nc.vector.memset(kv1_all[64:128, :], 0.0)
    nc.gpsimd.memset(kv1_all[64:65, :], 1.0)

    # ---- stage B: k1/v1 over full M + v1e transpose, SW-pipelined ----
    ppb = tc.tile_pool(name="ppb", bufs=3, space="PSUM")
    ppb_pool = ppb.__enter__()

    def emit_kv1(mt):
        ms = mt * MT
        src = src_tiles[mt // 2][:, mt % 2, :, :]
        ps1 = ppb_pool.tile([64, MT], F32, tag="ps1", name="ps1")
        nc.tensor.matmul(ps1, w8, src, start=True, stop=True,
                         perf_mode=mybir.MatmulPerfMode.DoubleRow)
        if mt % 2 == 0:
            nc.scalar.copy(out=kv1_all[0:64, ms:ms + MT], in_=ps1)
        else:
            nc.vector.tensor_copy(out=kv1_all[0:64, ms:ms + MT], in_=ps1)

    def emit_v1t(p):
        ms8 = p * 8
        psv = ppb_pool.tile([128, 8, 34], F32, tag="psv", name="psv")
        for j in range(8):
            mc = ms8 + j
            nc.tensor.matmul(psv[:, j, :], kv1_all[:, mc * MC:(mc + 1) * MC],
                             iv_ap, start=True, stop=True)
        nc.scalar.copy(out=v1t[:, ms8:ms8 + 8, 0:34], in_=psv)

    def emit_q1(nt):
        ns = nt * NT
        psq = ppb_pool.tile([D8, NT], F32, tag="ps1", name="psq")
        nc.tensor.matmul(psq, wq1(0), x_sb[:, 0, ns:ns + NT], start=True, stop=False)
        nc.tensor.matmul(psq, wq1(1), x_sb[:, 1, ns:ns + NT], start=False, stop=True)
        nc.vector.tensor_copy(out=q1[0:D8, ns:ns + NT], in_=psq)

    def emit_qh(h, nt, act):
        ns = nt * NT
        psq2 = ppb_pool.tile([128, NT], F32, tag="psv", name="psq2")
        nc.tensor.matmul(psq2, cht(h), q1[:, ns:ns + NT], start=True, stop=True)
        if act:
            nc.scalar.copy(out=qh_sb[:, h, ns:ns + NT], in_=psq2)
        else:
            nc.vector.tensor_copy(out=qh_sb[:, h, ns:ns + NT], in_=psq2)

    # fully interleaved stage B: kv1 stream + v1t transposes + the whole q
    # path, alternating copy engines so neither ACT nor DVE serializes it.
    emit_kv1(0)
    emit_kv1(1)
    emit_kv1(2)
    emit_kv1(3)
    emit_v1t(0)
    emit_kv1(4)
    emit_kv1(5)
    emit_v1t(1)
    emit_q1(0)
    emit_q1(1)
    emit_kv1(6)
    emit_kv1(7)
    emit_v1t(2)
    emit_qh(0, 0, act=True)
    emit_qh(0, 1, act=False)
    emit_v1t(3)
    emit_qh(1, 0, act=True)
    emit_qh(1, 1, act=False)
    emit_qh(2, 0, act=True)
    emit_qh(2, 1, act=False)
    emit_qh(3, 0, act=True)
    emit_qh(3, 1, act=False)

    ppb.__exit__(None, None, None)

    # ---- attention: flat pipeline over (nt, h, bi), lookahead 2 ----
    pps = ctx.enter_context(tc.tile_pool(name="pps", bufs=3, space="PSUM"))
    ppm = ctx.enter_context(tc.tile_pool(name="ppm", bufs=2, space="PSUM"))

    def emit_bridge_warm(dep_ap, n, off):
        # tiny DVE copy whose input is `dep_ap`: delays the following warm
        # MMs until that data exists, filling the PE hole right after it
        nc.vector.tensor_copy(out=wzb[0:1, off:off + 8], in_=dep_ap)
        emit_warm_mms(n)

    def emit_warm_mms(n, rhs=None):
        # dummy full-K matmuls: hold the HAM clock at 8/8 through thin spots.
        # Results are discarded.  Passing a just-written rhs tile delays
        # execution until that tile lands, so the warm MMs fill the
        # dependency-wait hole instead of running immediately.
        pw = ppm.tile([128, NT], F32, tag="pm", name="pwarm")
        r = wzb if rhs is None else rhs
        for _ in range(n):
            nc.tensor.matmul(pw[:, 0:r.shape[-1]], wza, r, start=True, stop=True)

    emit_warm_mms(3, rhs=qh_sb[:, 0, 0:NT])
    emit_warm_mms(3, rhs=qh_sb[:, 0, NT:2 * NT])

    def emit_scores(nt, h, bi):
        ns = nt * NT
        ps = pps.tile([128, 2, NT], F32, tag="ps", name="ps")
        for j in range(2):
            mc = bi * 2 + j
            nc.tensor.matmul(ps[:, j, :], kv1_all[:, mc * MC:(mc + 1) * MC],
                             qh_sb[:, h, ns:ns + NT], start=True, stop=True)
        e = ep.tile([128, 2, NT], FP8, tag="e", name="e")
        if dve_step(nt, h, bi):
            nc.vector.tensor_scalar(
                out=e[:, :, :].bitcast(I8), in0=ps, scalar1=float(EXP_A),
                scalar2=float(EXP_B), op0=ALU.mult, op1=ALU.add)
        else:
            nc.scalar.activation(out=e, in_=ps, func=AF.Exp, scale=0.125)
        return e

    def emit_norm(pm, h, ns, bridge=False):
        rec = nrm.tile([1, NT], F32, tag="rec", name="rec")
        nc.vector.reciprocal_approx_fast(out=rec, in_=pm[0:1, :])
        if bridge:
            emit_bridge_warm(rec[0:1, 0:8].bitcast(BF16)[0:1, 0:8], 3, 0)
        bc = nrm.tile([33, NT], F32, tag="bc", name="bc")
        nc.gpsimd.partition_broadcast(bc, rec)
        nc.vector.tensor_mul(out=msg_sb[0:33, h, ns:ns + NT],
                             in0=pm[0:33, :], in1=bc)


    def make_merge_pieces(nt):
        # nt0's merge+mlp split into 4 pieces, one per nt1 head boundary.
        # Each allocates exactly one 1-bank ppm tile, interleaving cleanly
        # with the pm rotation so the scores pipeline is never disturbed.
        ns = nt * NT
        st = {}

        def p_ct(ct):
            def f():
                t = ppm.tile([128, NT], F32, tag="pm", name="psm2n")
                for h in range(H):
                    nc.tensor.matmul(t, wmm(h, ct), msg_sb[:, h, ns:ns + NT],
                                     start=(h == 0), stop=(h == H - 1))
                nc.vector.tensor_copy(out=mm_sb[:, ct, ns:ns + NT], in_=t)
            return f

        def p_psh():
            t = ppm.tile([128, NT], F32, tag="pm", name="pshn")
            st["ph"] = t
            psh = t[0:TD8, :]
            nc.tensor.matmul(psh, wp1x(0), x_sb[:, 0, ns:ns + NT], start=True, stop=False)
            nc.tensor.matmul(psh, wp1x(1), x_sb[:, 1, ns:ns + NT], start=False, stop=False)
            nc.tensor.matmul(psh, wp1m(0), mm_sb[:, 0, ns:ns + NT], start=False, stop=False)
            nc.tensor.matmul(psh, wp1m(1), mm_sb[:, 1, ns:ns + NT], start=False, stop=True)
            nc.scalar.activation(out=h1[0:TD8, ns:ns + NT], in_=psh, func=AF.Relu,
                                 bias=be1_ap)

        def p_out():
            t = ppm.tile([128, NT], F32, tag="pm", name="pson")
            for ct, tt in ((0, t), (1, st["ph"])):
                nc.tensor.matmul(tt[:, :], wp2(ct), h1[:, ns:ns + NT],
                                 start=True, stop=True)
                nc.vector.tensor_copy(out=out_sb[:, ct, ns:ns + NT], in_=tt)
                nc.sync.dma_start(out=out_d[ct, :, ns:ns + NT],
                                  in_=out_sb[:, ct, ns:ns + NT])

        return [p_ct(0), p_ct(1), p_psh, p_out]

    merge_pieces = []

    def emit_merge_mlp(nt, warm=False):
        # the tail merge runs half-width (256-col) chains for latency;
        # mid-attention merges run full-width to minimize pps disturbance
        nhalf = 2 if warm else 1
        HT = NT // nhalf
        # bridge: a tiny DVE copy dependent on the last norm makes the
        # following warm MMs fire exactly when the PE would go idle
        nc.vector.tensor_copy(out=wzb[0:33, 0:8],
                              in_=msg_sb[:, H - 1, nt * NT:nt * NT + 8])
        emit_warm_mms(5)
        for half in range(nhalf):
            ns = nt * NT + half * HT
            psm2 = pps.tile([128, 2, HT], F32, tag="ps", name="psm2")
            for ct in range(2):
                for h in range(H):
                    nc.tensor.matmul(psm2[:, ct, :], wmm(h, ct),
                                     msg_sb[:, h, ns:ns + HT],
                                     start=(h == 0), stop=(h == H - 1))
                nc.vector.tensor_copy(out=mm_sb[:, ct, ns:ns + HT],
                                      in_=psm2[:, ct, :])
                if warm:
                    emit_warm_mms(2, rhs=mm_sb[:, ct, ns:ns + HT])
            phb = pps.tile([128, 2, HT], F32, tag="ps", name="phb")
            psh = phb[0:TD8, 0, :]
            nc.tensor.matmul(psh, wp1x(0), x_sb[:, 0, ns:ns + HT], start=True, stop=False)
            nc.tensor.matmul(psh, wp1x(1), x_sb[:, 1, ns:ns + HT], start=False, stop=False)
            nc.tensor.matmul(psh, wp1m(0), mm_sb[:, 0, ns:ns + HT], start=False, stop=False)
            nc.tensor.matmul(psh, wp1m(1), mm_sb[:, 1, ns:ns + HT], start=False, stop=True)
            nc.scalar.activation(out=h1[0:TD8, ns:ns + HT], in_=psh, func=AF.Relu,
                                 bias=be1_ap)
            if warm:
                emit_bridge_warm(h1[0:1, ns:ns + 8].bitcast(BF16)[0:1, 0:8],
                                 2, 8 + 8 * half)
            for ct in range(2):
                pso = phb[:, 1 - ct, :]
                nc.tensor.matmul(pso, wp2(ct), h1[:, ns:ns + HT], start=True, stop=True)
                nc.vector.tensor_copy(out=out_sb[:, ct, ns:ns + HT], in_=pso)
                nc.sync.dma_start(out=out_d[ct, :, ns:ns + HT],
                                  in_=out_sb[:, ct, ns:ns + HT])

    seq = [(nt, h, bi) for nt in range(NTILES) for h in range(H)
           for bi in range(NSTEP)]
    pend = {}
    pm = None

    def emit_msg(idx):
        nonlocal pm
        nt, h, bi = seq[idx]
        if bi == 0:
            pm = ppm.tile([33, NT], F32, tag="pm", name="pm")
        e = pend.pop(idx)
        nc.tensor.matmul(pm, v1t[:, 2 * bi: 2 * bi + 2, 0:33], e,
                         start=(bi == 0), stop=(bi == NSTEP - 1),
                         perf_mode=mybir.MatmulPerfMode.DoubleRow)
        if bi == NSTEP - 1:
            emit_norm(pm, h, nt * NT,
                      bridge=(nt == NTILES - 1 and h == H - 1))
            if nt == 0 and h == H - 1:
                merge_pieces[:] = make_merge_pieces(0)
            elif nt == NTILES - 1:
                merge_pieces[h]()
                if h == H - 1:
                    emit_merge_mlp(nt, warm=True)

    LOOK = 2
    for idx, step in enumerate(seq):
        pend[idx] = emit_scores(*step)
        if idx >= LOOK:
            emit_msg(idx - LOOK)
    for idx in range(len(seq) - LOOK, len(seq)):
        emit_msg(idx)


def build_program():
    nc = bacc.Bacc("TRN2", target_bir_lowering=False, debug=False)
    io = {}
    io["x_chunk"] = nc.dram_tensor("x_chunk", [128, 2, NCHUNK], BF16,
                                   kind="ExternalInput").ap()
    io["source_b"] = nc.dram_tensor("source_b", [128, MTILES, 2, MT], FP8,
                                    kind="ExternalInput").ap()
    io["c8"] = nc.dram_tensor("c8", [128, 2, 64], FP8,
                              kind="ExternalInput").ap()
    io["ce"] = nc.dram_tensor("ce", [128, CE_END], BF16, kind="ExternalInput").ap()
    io["cb"] = nc.dram_tensor("cb", [128, CB_END], BF16, kind="ExternalInput").ap()
    io["cf"] = nc.dram_tensor("cf", [128, CF_END], F32R, kind="ExternalInput").ap()
    io["out_chunk"] = nc.dram_tensor(
        "out_chunk", [2, 128, NCHUNK], F32, kind="ExternalOutput").ap()
    from contextlib import ExitStack
    with tile.TileContext(nc) as tc, ExitStack() as ctx:
        build_body(ctx, tc, io)
    nc.compile()
    return nc


def prep_weights(i):
    """Host-side folds; see module docstring."""
    import ml_dtypes
    bf = ml_dtypes.bfloat16
    f = np.float32
    a = {k: np.asarray(v, dtype=f) for k, v in i.items()}
    # head-contiguous channel permutation: c' = h*64+d  <- c = 4*d+h
    perm = (np.arange(H)[:, None] + H * np.arange(DIM)[None, :]).reshape(-1)

    def w1t(w):       # [D8, D] -> [128, 2*D8] (chunk-major)
        return np.ascontiguousarray(
            w.T.reshape(2, 128, D8).swapaxes(0, 1).reshape(128, 2 * D8))

    cep = np.zeros((128, CE_END), np.float64)
    cbp = np.zeros((128, CB_END), np.float64)
    cfp = np.zeros((128, CF_END), np.float64)

    cep[:, CE_WQ:CE_WQ + 64] = w1t(a["Wq1"])
    c8 = np.zeros((128, 2, 64), np.float32)
    c8[:, :, 0:32] = w1t(a["Wk1"]).reshape(128, 2, D8)
    c8[:, :, 32:64] = w1t(a["Wv1"]).reshape(128, 2, D8)

    # cht: C_h = Ak'_h @ Aq'_h^T with first-proj biases folded into ones-rows
    wq2e = np.concatenate([a["Wq2"][perm].T, a["bq2"][perm][None, :]], 0)  # [33, 256]
    wk2e = np.concatenate([a["Wk2"][perm].T, a["bk2"][perm][None, :]], 0)
    for h in range(H):
        hs = slice(h * DIM, (h + 1) * DIM)
        Ak = wk2e[:, hs].astype(np.float64).copy()      # [33, 64]
        Aq = wq2e[:, hs].astype(np.float64).copy()
        Ak[32, :] += a["bk1"].astype(np.float64) @ Ak[0:32, :]
        Aq[32, :] += a["bq1"].astype(np.float64) @ Aq[0:32, :]
        C = Ak @ Aq.T                                   # [33(k-space), 33(q-space)]
        # cht[kq, h, p]: p<32 -> C[p, kq]; p=64 -> C[32, kq]
        blk = np.zeros((33, 128), np.float64)
        blk[:, 0:32] = C[0:32, :].T
        blk[:, 64] = C[32, :]
        cep[0:33, CE_CHT + h * 128: CE_CHT + (h + 1) * 128] = blk

    # iv: [65, 34] selector: col 0 <- ones row (64), col 1+j <- v1 row (32+j)
    iv = np.zeros((65, 34), np.float64)
    iv[64, 0] = 1.0
    for j in range(32):
        iv[32 + j, 1 + j] = 1.0
    cep[0:65, CE_IV:CE_IV + 34] = iv

    # wmm: mm = sum_h Wm2 @ [bias_h | W_h] @ msgv_h  (+ Wm2@bm1 + bm2 on h=0 col0)
    Wm2 = a["Wm2"].astype(np.float64)                   # [256, 32]
    for h in range(H):
        hs = perm[h * DIM:(h + 1) * DIM]
        Wm1p = a["Wm1"][:, hs].astype(np.float64)       # [32, 64]
        Wv2p = a["Wv2"][hs, :].astype(np.float64)       # [64, 32]
        bv2p = a["bv2"][hs].astype(np.float64)          # [64]
        W_h = Wm1p @ Wv2p                               # [32, 32]
        bias_h = Wm1p @ (bv2p + Wv2p @ a["bv1"].astype(np.float64))
        if h == 0:
            bias_h = bias_h + a["bm1"].astype(np.float64)
        Mfull = np.concatenate([bias_h[:, None], W_h], 1)   # [32, 33] (m1-space)
        M2 = Wm2 @ Mfull                                # [256, 33]
        if h == 0:
            M2[:, 0] += a["bm2"].astype(np.float64)
        for ct in range(2):
            cbp[0:33, CB_WMM + (h * 2 + ct) * 128: CB_WMM + (h * 2 + ct + 1) * 128] = \
                M2[ct * 128:(ct + 1) * 128, :].T

    # BN scale folds into Wp1; shift+bias into relu bias
    g1s = (a["g1"] / np.sqrt(np.float64(1.0) + np.float64(BN_EPS))).astype(np.float64)
    wp1x = a["Wp1"][:, 0:D].astype(np.float64) * g1s[:, None]     # [64, 256]
    wp1m = a["Wp1"][:, D:TD].astype(np.float64) * g1s[:, None]
    cbp[:, CB_WP1X:CB_WP1X + 128] = \
        wp1x.T.reshape(2, 128, TD8).swapaxes(0, 1).reshape(128, 128)
    cbp[:, CB_WP1M:CB_WP1M + 128] = \
        wp1m.T.reshape(2, 128, TD8).swapaxes(0, 1).reshape(128, 128)
    cfp[0:TD8, CF_BE1] = a["be1"].astype(np.float64) + g1s * a["bp1"].astype(np.float64)

    wp2t = np.concatenate(
        [a["Wp2"].T.reshape(TD8, 2, 128), a["bp2"].reshape(1, 2, 128)], 0)  # [65,2,128]
    cfp[0:TD8 + 1, CF_WP2:CF_WP2 + 256] = wp2t.reshape(TD8 + 1, 256)

    return {"ce": np.ascontiguousarray(cep.astype(f).astype(bf)),
            "cb": np.ascontiguousarray(cbp.astype(f).astype(bf)),
            "cf": np.ascontiguousarray(cfp.astype(f)),
            "c8": np.ascontiguousarray(c8.astype(ml_dtypes.float8_e4m3))}


_NC_CACHE = None


def _get_nc():
    global _NC_CACHE
    if _NC_CACHE is None:
        _NC_CACHE = build_program()
    return _NC_CACHE


def make_in_maps(inputs):
    import ml_dtypes
    w = prep_weights(inputs)
    x = np.ascontiguousarray(np.asarray(inputs["x"], np.float32))
    src = np.ascontiguousarray(np.asarray(inputs["source"], np.float32))
    in_maps = []
    for c in range(NCORES):
        b, ns = c // 4, (c % 4) * NCHUNK
        m = dict(w)
        m["x_chunk"] = np.ascontiguousarray(
            x[b].reshape(2, 128, N)[:, :, ns:ns + NCHUNK].transpose(1, 0, 2)
            ).astype(ml_dtypes.bfloat16)
        m["source_b"] = np.ascontiguousarray(
            src[b].reshape(2, 128, MTILES, MT).transpose(1, 2, 0, 3)).astype(
            ml_dtypes.float8_e4m3)
        in_maps.append(m)
    return in_maps


def assemble_out(results):
    out = np.empty((B, D, N), np.float32)
    for c in range(NCORES):
        b, ns = c // 4, (c % 4) * NCHUNK
        out[b].reshape(2, 128, N)[:, :, ns:ns + NCHUNK] = (
            results[c]["out_chunk"])
    return out


def kernel(**inputs):
    nc = _get_nc()
    res = bass_utils.run_bass_kernel_spmd(
        nc, make_in_maps(inputs), core_ids=list(range(NCORES)))
    return assemble_out(res.results)
